# revision 40
# baseline (speedup 1.0000x reference)
"""Trainium2 Bass kernel for nn_AttentionLayer (B=2, S=2048, HIDDEN=3072, 32 heads,
head_dim=96, RoPE, causal, source quirk: v = k pre-RoPE; v-projection unused).

Sharding: tensor-parallel over heads — 4 heads per core on 8 cores. Each core:
  - QK projection for its heads (transposed layouts, host-pretransposed weights)
  - RoPE on DVE (host-built sign-folded cos/sin tables + partition-swap copies)
  - causal attention with scores computed transposed S_t[k,q]; softmax without
    max-subtraction (scores are O(1) by construction); softmax denominator folded
    into the attn@v matmul via an appended ones-column in v
  - column-sharded o_proj producing a bf16 partial [3072, B*S], with the
    o_proj of each group software-pipelined 3 groups behind its attention
Host sums the 8 partials (the tensor-parallel all-reduce) and transposes back.
Measured: ~640-650us HW exec (neuron-profile), rel err ~0.0066 vs the bf16
jax reference (PE-engine roofline for the bf16 matmul work is ~512us/core).

Self-contained: hardcodes shapes; no reads of /root/problem/*.
"""

import math
import os
import sys

import numpy as np

sys.path.insert(0, "/opt/trn_rl_repo")

import ml_dtypes

BF16 = ml_dtypes.bfloat16

HEADS = 32
HIDDEN = 3072
HD = 96  # head dim
ROPE_THETA = 10000.0
N_CORES = 8
HPC = HEADS // N_CORES  # 4 heads per core
DPC = HPC * HD  # 384 dcols per core


# ---------------------------------------------------------------- host prep

def _rope_tables(S, T):
    """C,S tables [96, T] (T = B*S, position repeats per batch), sign-folded for
    the 'q*C + swap48(q)*S' formulation. bf16 to match the reference's cast."""
    inv_freq = 1.0 / (ROPE_THETA ** (np.arange(0, HD, 2, dtype=np.float64) / HD))
    pos = np.arange(S, dtype=np.float64)
    ang = pos[:, None] * inv_freq[None, :]  # [S, 48]
    cos = np.cos(ang).astype(np.float32)
    sin = np.sin(ang).astype(np.float32)
    nrep = T // S
    C = np.zeros((HD, T), dtype=np.float32)
    Sg = np.zeros((HD, T), dtype=np.float32)
    for r in range(nrep):
        sl = slice(r * S, (r + 1) * S)
        C[:48, sl] = cos.T
        C[48:, sl] = cos.T
        Sg[:48, sl] = -sin.T
        Sg[48:, sl] = sin.T
    return C.astype(BF16), Sg.astype(BF16)


def _masks(QT=512, KC=128):
    """Diagonal-block keep masks [QT//KC, KC, QT]: mask[j,kk,qq] = qq >= KC*j+kk."""
    nj = QT // KC
    qq = np.arange(QT)[None, None, :]
    kk = np.arange(KC)[None, :, None]
    j = np.arange(nj)[:, None, None]
    return (qq >= KC * j + kk).astype(BF16)


def host_inputs_for_core(core, x_t, w_qkv, w_o, C, Sg, masks):
    c = core
    wq = w_qkv[DPC * c: DPC * (c + 1)]                      # [384, 3072]
    wk = w_qkv[HIDDEN + DPC * c: HIDDEN + DPC * (c + 1)]    # [384, 3072]
    wqk_t = np.concatenate([wq, wk], 0).T.astype(BF16)   # [3072, 768]
    # pre-tile for contiguous SBUF loads: [p, dcol, hh, c]
    wqk_r = np.ascontiguousarray(
        wqk_t.reshape(24, 128, 6, 128).transpose(1, 2, 0, 3))
    w_o_t = w_o[:, DPC * c: DPC * (c + 1)].T.astype(BF16)  # [384, 3072]
    w_o_r = np.ascontiguousarray(w_o_t.reshape(3, 128, HIDDEN).transpose(1, 0, 2))
    return {
        "x_t": x_t,
        "wqk_t": wqk_r,
        "w_o_t": w_o_r,
        "cos_t": C,
        "sin_t": Sg,
        "masks": np.ascontiguousarray(masks[0, :, 0:128]),
    }


def retile_x(x_t):
    """[3072, T] -> [T//512, 128, 24, 512] so each 512-token projection tile
    loads with one contiguous run per partition."""
    H, T = x_t.shape
    nt = T // 512
    # x_t[hh*128+p, t*512+c] -> out[t, p, hh, c]
    v = x_t.reshape(24, 128, nt, 512)
    return np.ascontiguousarray(v.transpose(2, 1, 0, 3))


# ---------------------------------------------------------------- device graph

def build_nc(S=2048, B=2, out_f32=False):
    import concourse.bass as bass
    import concourse.mybir as mybir
    import concourse.tile as tile
    from concourse import bacc
    from concourse.bass import ts, ds
    from concourse.masks import make_identity
    from contextlib import ExitStack

    dt = mybir.dt
    T = B * S                 # tokens total
    NT = T // 512             # 512-token tiles for proj
    HCH = HIDDEN // 128       # 24 hidden chunks
    NQT = S // 512            # q-tiles per batch
    NKC = S // 128            # k-chunks per batch
    SCALE = 1.0 / math.sqrt(HD)
    out_dt = dt.float32 if out_f32 else dt.bfloat16

    nc = bacc.Bacc("TRN2", target_bir_lowering=False, debug=False)

    x_t = nc.dram_tensor("x_t", [T // 512, 128, HCH, 512], dt.bfloat16,
                     kind="ExternalInput").ap()
    wqk = nc.dram_tensor("wqk_t", [128, 2 * DPC // 128, HCH, 128], dt.bfloat16,
                     kind="ExternalInput").ap()
    wot = nc.dram_tensor("w_o_t", [128, DPC // 128, HIDDEN], dt.bfloat16,
                     kind="ExternalInput").ap()
    cosd = nc.dram_tensor("cos_t", [HD, T], dt.bfloat16, kind="ExternalInput").ap()
    sind = nc.dram_tensor("sin_t", [HD, T], dt.bfloat16, kind="ExternalInput").ap()
    maskd = nc.dram_tensor("masks", [128, 128], dt.bfloat16, kind="ExternalInput").ap()
    outd = nc.dram_tensor("out", [HIDDEN, T], out_dt, kind="ExternalOutput").ap()

    with tile.TileContext(nc) as tc, ExitStack() as stk:
        # ------- persistent tiles (live across phases)
        pers = stk.enter_context(tc.tile_pool(name="pers", bufs=1))
        q_h = [pers.tile([128, T], dt.bfloat16, tag=f"q{h}", name=f"q{h}") for h in range(HPC)]
        k_h = [pers.tile([128, T], dt.bfloat16, tag=f"k{h}", name=f"k{h}") for h in range(HPC)]
        v1_h = [pers.tile([128, NKC * B, HD + 1], dt.bfloat16, tag=f"v{h}", name=f"v{h}")
                for h in range(HPC)]

        for h in range(HPC):
            nc.gpsimd.memset(q_h[h][HD:128, :], 0.0)
            nc.gpsimd.memset(k_h[h][HD:128, :], 0.0)
            nc.gpsimd.memset(v1_h[h][:, :, HD:HD + 1], 1.0)

        # ------- phase 1+2 fused: QK projection, v-transposes, RoPE —
        # interleaved per 512-token tile so PE stays dense and warm
        with tc.tile_pool(name="projw", bufs=1) as projw, \
             tc.tile_pool(name="xload", bufs=2) as xload, \
             tc.tile_pool(name="projsc", bufs=3) as projsc, \
             tc.tile_pool(name="swp", bufs=4) as swp, \
             tc.tile_pool(name="projps", bufs=2, space="PSUM") as projps, \
             tc.tile_pool(name="tps", bufs=2, space="PSUM") as tps:
            qkw_c = []
            for dcol in range(2 * DPC // 128):
                qt_ = projw.tile([128, HCH, 128], dt.bfloat16, tag=f"qkw{dcol}",
                                 name=f"qkw{dcol}")
                nc.scalar.dma_start(qt_[:], wqk[:, dcol])
                qkw_c.append(qt_)
            C_sb = projw.tile([HD, T], dt.bfloat16, tag="cos", name="C_sb")
            S_sb = projw.tile([HD, T], dt.bfloat16, tag="sin", name="S_sb")

            ident = projw.tile([128, 128], dt.bfloat16, name="ident")
            make_identity(nc, ident)

            # dcol block -> list of (head-tensor, dst_row0, src_row0, nrows)
            def segs(block):
                out = []
                r0, r1 = 128 * block, 128 * (block + 1)
                for side, dest in ((0, q_h), (1, k_h)):
                    for h in range(HPC):
                        h0 = side * DPC + h * HD
                        lo, hi = max(r0, h0), min(r1, h0 + HD)
                        if lo < hi:
                            out.append((dest[h], lo - h0, lo - r0, hi - lo))
                return out

            def emit_vt_rope(t):
                tsl = ds(t * 512, 512)
                if t == 0:
                    # C/S tables aren't needed until the first RoPE — keep them
                    # out of the startup DMA flood
                    nc.scalar.dma_start(C_sb[:], cosd)
                    nc.scalar.dma_start(S_sb[:], sind)
                # v1[token, kc, d] = k_h[d, kc*128+token] (pre-RoPE), this t-slice
                for h in range(HPC):
                    for kc in range(4 * t, 4 * t + 4):
                        tp = tps.tile([128, 128], dt.bfloat16, tag="tp", name="tp")
                        nc.tensor.transpose(tp, k_h[h][:, ts(kc, 128)], ident)
                        nc.vector.tensor_copy(v1_h[h][:, kc, 0:HD], tp[:, 0:HD])
                # RoPE this t-slice in place: v = v*C + swap48(v)*S
                for h in range(HPC):
                    for tile_ in (k_h[h], q_h[h]):
                        sw = swp.tile([HD, 512], dt.bfloat16, tag="sw", name="sw")
                        nc.sync.dma_start(sw[0:48, :], tile_[48:HD, tsl])
                        nc.sync.dma_start(sw[48:HD, :], tile_[0:48, tsl])
                        nc.vector.tensor_tensor(
                            tile_[0:HD, tsl], tile_[0:HD, tsl], C_sb[:, tsl],
                            mybir.AluOpType.mult)
                        nc.vector.tensor_tensor(
                            sw[:], sw[:], S_sb[:, tsl], mybir.AluOpType.mult)
                        nc.vector.tensor_tensor(
                            tile_[0:HD, tsl], tile_[0:HD, tsl], sw[:],
                            mybir.AluOpType.add)

            for t in range(NT):
                tsl = ds(t * 512, 512)
                x_g = []
                for g in range(3):
                    xg = xload.tile([128, 8, 512], dt.bfloat16, tag=f"x{g}",
                                    name=f"x{g}")
                    eng = nc.scalar if (t == 0 and g == 1) else nc.sync
                    eng.dma_start(xg[:], x_t[t, :, ts(g, 8), :])
                    x_g.append(xg)
                for dcol in range(2 * DPC // 128):
                    ps = projps.tile([128, 512], dt.float32, tag="pp", name="ps")
                    for hh in range(HCH):
                        nc.tensor.matmul(
                            ps, lhsT=qkw_c[dcol][:, hh, :],
                            rhs=x_g[hh // 8][:, hh % 8, :],
                            start=(hh == 0), stop=(hh == HCH - 1))
                    sc = projsc.tile([128, 512], dt.bfloat16, tag="sc", name="sc")
                    nc.scalar.copy(sc, ps)
                    for dest, d0, s0, n in segs(dcol):
                        nc.sync.dma_start(dest[d0:d0 + n, tsl], sc[s0:s0 + n, :])
                # transposes/RoPE of the PREVIOUS tile — its scatters finished a
                # full tile ago, so the PE stream never stalls on them
                if t > 0:
                    emit_vt_rope(t - 1)
            emit_vt_rope(NT - 1)

        # ------- phase 3+4: attention + o_proj per (batch, q-tile)
        with tc.tile_pool(name="attw", bufs=1) as attw, \
             tc.tile_pool(name="pbuf", bufs=6) as pbuf, \
             tc.tile_pool(name="nrm", bufs=3) as nrm, \
             tc.tile_pool(name="unp", bufs=8) as unp, \
             tc.tile_pool(name="atq", bufs=4) as atq, \
             tc.tile_pool(name="osb", bufs=3) as osb, \
             tc.tile_pool(name="nscr", bufs=3, space="DRAM") as nscr, \
             tc.tile_pool(name="sps", bufs=4, space="PSUM") as sps, \
             tc.tile_pool(name="ops", bufs=2, space="PSUM") as ops, \
             tc.tile_pool(name="ops2", bufs=2, space="PSUM") as ops2:
            w_o_sb = attw.tile([128, DPC // 128, HIDDEN], dt.bfloat16, tag="wo",
                               name="w_o_sb")
            nc.scalar.dma_start(w_o_sb[:], wot)
            mask_sb = attw.tile([128, 128], dt.bfloat16, tag="mk", name="mask_sb")
            nc.scalar.dma_start(mask_sb[:], maskd)

            def emit_oproj(g):
                gb, gqt, gat = g
                for oc in range(HIDDEN // 128):
                    ps2 = ops2.tile([128, 512], dt.float32, tag="ops2", name="ps2")
                    for ic in range(DPC // 128):
                        nc.tensor.matmul(
                            ps2, lhsT=w_o_sb[:, ic, ts(oc, 128)],
                            rhs=gat[:, ic, :],
                            start=(ic == 0), stop=(ic == DPC // 128 - 1))
                    ot = osb.tile([128, 512], out_dt, tag="ot", name="ot")
                    nc.vector.tensor_copy(ot, ps2)
                    nc.sync.dma_start(
                        outd[ts(oc, 128), ds(gb * S + gqt * 512, 512)], ot)

            pending = []
            for qt in range(NQT):
                for b in range(B):
                    at_qt = atq.tile([128, DPC // 128, 512], dt.bfloat16, tag="atq",
                                     name="at_qt")
                    sums4 = nscr.tile([HPC, 512], dt.bfloat16, tag="sums4",
                                      name="sums4")
                    un_h = []
                    for h in range(HPC):
                        q_slice = q_h[h][:, ds(b * S + qt * 512, 512)]
                        nlive = 4 * (qt + 1)
                        o_ps = ops.tile([128, 512], dt.float32, tag="ops", name="o_ps")
                        # off-diagonal chunks (full 512-wide)
                        for kc in range(4 * qt):
                            s_ps = sps.tile([128, 512], dt.float32, tag="sps",
                                            name="s_ps")
                            nc.tensor.matmul(
                                s_ps,
                                lhsT=k_h[h][:, ds(b * S + kc * 128, 128)],
                                rhs=q_slice, start=True, stop=True)
                            p_sb = pbuf.tile([128, 512], dt.bfloat16, tag="p",
                                             name="p_sb")
                            nc.scalar.activation(
                                p_sb[:], s_ps[:], mybir.ActivationFunctionType.Exp,
                                scale=SCALE)
                            nc.tensor.matmul(
                                o_ps[0:HD + 1, :],
                                lhsT=v1_h[h][:, b * NKC + kc, :],
                                rhs=p_sb,
                                start=(kc == 0), stop=False)
                        # diagonal chunks, width-restricted to valid columns
                        for j in range(4):
                            kc = 4 * qt + j
                            w = 512 - 128 * j
                            s_ps = sps.tile([128, 512], dt.float32, tag="sps",
                                            name="s_psd")
                            nc.tensor.matmul(
                                s_ps[:, 0:w],
                                lhsT=k_h[h][:, ds(b * S + kc * 128, 128)],
                                rhs=q_slice[:, 128 * j:], start=True, stop=True)
                            p_sb = pbuf.tile([128, 512], dt.bfloat16, tag="p",
                                             name="p_sbd")
                            nc.scalar.activation(
                                p_sb[:, 0:w], s_ps[:, 0:w],
                                mybir.ActivationFunctionType.Exp, scale=SCALE)
                            nc.vector.tensor_tensor(
                                p_sb[:, 0:128], p_sb[:, 0:128],
                                mask_sb[:, :], mybir.AluOpType.mult)
                            nc.tensor.matmul(
                                o_ps[0:HD + 1, 128 * j:],
                                lhsT=v1_h[h][:, b * NKC + kc, :],
                                rhs=p_sb[:, 0:w],
                                start=(kc == 0), stop=(kc == nlive - 1))
                        # copy out unnormalized attn + sums row (releases PSUM fast)
                        un = unp.tile([HD + 1, 512], dt.bfloat16, tag="un",
                                      name=f"un{h}")
                        nc.vector.tensor_copy(un[0:64, :], o_ps[0:64, :])
                        nc.scalar.copy(un[64:HD + 1, :], o_ps[64:HD + 1, :])
                        un_h.append(un)
                        nc.sync.dma_start(sums4[h:h + 1, :], un[HD:HD + 1, :])
                    # one reciprocal for all 4 heads' sums, then normalize each
                    rb4b = nrm.tile([HPC, 512], dt.bfloat16, tag="rb4b", name="rb4b")
                    nc.sync.dma_start(rb4b[:], sums4[:])
                    rb4 = nrm.tile([HPC, 512], dt.bfloat16, tag="rb4", name="rb4")
                    with nc.allow_low_precision(
                            reason="recip in bf16: scale-only error ~0.4%"):
                        nc.vector.reciprocal(rb4[:], rb4b[:])
                    rec4 = nscr.tile([HPC, 512], dt.bfloat16, tag="rec4", name="rec4")
                    nc.sync.dma_start(rec4[:], rb4[:])
                    for h in range(HPC):
                        rb = nrm.tile([HD, 512], dt.bfloat16, tag="rb", name="rb")
                        nc.sync.dma_start(
                            rb[:], rec4[h:h + 1, :].to_broadcast([HD, 512]))
                        at = nrm.tile([HD, 512], dt.bfloat16, tag="at", name="at")
                        nc.vector.tensor_tensor(
                            at, un_h[h][0:HD, :], rb[:], mybir.AluOpType.mult)
                        r0 = h * HD
                        while r0 < (h + 1) * HD:
                            blk = r0 // 128
                            n = min(128 * (blk + 1), (h + 1) * HD) - r0
                            nc.sync.dma_start(
                                at_qt[r0 - 128 * blk: r0 - 128 * blk + n, blk, :],
                                at[r0 - h * HD: r0 - h * HD + n, :])
                            r0 += n
                    # o_proj pipelined two groups behind attention so each
                    # group's norm chain has two groups of PE work as cover
                    pending.append((b, qt, at_qt))
                    if len(pending) > 3:
                        emit_oproj(pending.pop(0))
            for g in pending:
                emit_oproj(g)

    return nc


# ---------------------------------------------------------------- entry point

_NC_CACHE = {}


def _get_nc(S, B):
    key = (S, B)
    if key not in _NC_CACHE:
        nc = build_nc(S=S, B=B)
        nc.finalize()
        _NC_CACHE[key] = nc
    return _NC_CACHE[key]


def kernel(x, w_qkv, w_o, _trace=False):
    from concourse import bass_utils

    B, S, _ = x.shape
    T = B * S
    xf = np.asarray(x).reshape(T, HIDDEN)
    x_t = retile_x(np.ascontiguousarray(xf.T).astype(BF16))
    w_qkv = np.asarray(w_qkv).astype(BF16)
    w_o = np.asarray(w_o).astype(BF16)
    C, Sg = _rope_tables(S, T)
    masks = _masks()

    in_maps = [host_inputs_for_core(c, x_t, w_qkv, w_o, C, Sg, masks)
               for c in range(N_CORES)]

    nc = _get_nc(S, B)
    res = bass_utils.run_bass_kernel_spmd(
        nc, in_maps, core_ids=list(range(N_CORES)), trace=_trace)

    total = np.zeros((HIDDEN, T), dtype=np.float32)
    for c in range(N_CORES):
        total += np.asarray(res.results[c]["out"], dtype=np.float32)
    out = total.T.reshape(B, S, HIDDEN).astype(BF16)
    if _trace:
        return out, res
    return out


# revision 41
# speedup vs baseline: 1.0109x; 1.0109x over previous
"""Trainium2 Bass kernel for nn_AttentionLayer (B=2, S=2048, HIDDEN=3072, 32 heads,
head_dim=96, RoPE, causal, source quirk: v = k pre-RoPE; v-projection unused).

Sharding: tensor-parallel over heads — 4 heads per core on 8 cores. Each core:
  - QK projection for its heads (transposed layouts, host-pretransposed weights)
  - RoPE on DVE (host-built sign-folded cos/sin tables + partition-swap copies)
  - causal attention with scores computed transposed S_t[k,q]; softmax without
    max-subtraction (scores are O(1) by construction); softmax denominator folded
    into the attn@v matmul via an appended ones-column in v
  - column-sharded o_proj producing a bf16 partial [3072, B*S], with the
    o_proj of each group software-pipelined 3 groups behind its attention
Host sums the 8 partials (the tensor-parallel all-reduce) and transposes back.
Measured: ~640-650us HW exec (neuron-profile), rel err ~0.0066 vs the bf16
jax reference (PE-engine roofline for the bf16 matmul work is ~512us/core).

Self-contained: hardcodes shapes; no reads of /root/problem/*.
"""

import math
import os
import sys

import numpy as np

sys.path.insert(0, "/opt/trn_rl_repo")

import ml_dtypes

BF16 = ml_dtypes.bfloat16

HEADS = 32
HIDDEN = 3072
HD = 96  # head dim
ROPE_THETA = 10000.0
N_CORES = 8
HPC = HEADS // N_CORES  # 4 heads per core
DPC = HPC * HD  # 384 dcols per core


# ---------------------------------------------------------------- host prep

def _rope_tables(S, T):
    """C,S tables [96, T] (T = B*S, position repeats per batch), sign-folded for
    the 'q*C + swap48(q)*S' formulation. bf16 to match the reference's cast."""
    inv_freq = 1.0 / (ROPE_THETA ** (np.arange(0, HD, 2, dtype=np.float64) / HD))
    pos = np.arange(S, dtype=np.float64)
    ang = pos[:, None] * inv_freq[None, :]  # [S, 48]
    cos = np.cos(ang).astype(np.float32)
    sin = np.sin(ang).astype(np.float32)
    nrep = T // S
    C = np.zeros((HD, T), dtype=np.float32)
    Sg = np.zeros((HD, T), dtype=np.float32)
    for r in range(nrep):
        sl = slice(r * S, (r + 1) * S)
        C[:48, sl] = cos.T
        C[48:, sl] = cos.T
        Sg[:48, sl] = -sin.T
        Sg[48:, sl] = sin.T
    return C.astype(BF16), Sg.astype(BF16)


def _masks(QT=512, KC=128):
    """Diagonal-block keep masks [QT//KC, KC, QT]: mask[j,kk,qq] = qq >= KC*j+kk."""
    nj = QT // KC
    qq = np.arange(QT)[None, None, :]
    kk = np.arange(KC)[None, :, None]
    j = np.arange(nj)[:, None, None]
    return (qq >= KC * j + kk).astype(BF16)


def host_inputs_for_core(core, x_t, w_qkv, w_o, C, Sg, masks):
    c = core
    wq = w_qkv[DPC * c: DPC * (c + 1)]                      # [384, 3072]
    wk = w_qkv[HIDDEN + DPC * c: HIDDEN + DPC * (c + 1)]    # [384, 3072]
    wqk_t = np.concatenate([wq, wk], 0).T.astype(BF16)   # [3072, 768]
    # pre-tile for contiguous SBUF loads: [p, dcol, hh, c]
    wqk_r = np.ascontiguousarray(
        wqk_t.reshape(24, 128, 6, 128).transpose(1, 2, 0, 3))
    w_o_t = w_o[:, DPC * c: DPC * (c + 1)].T.astype(BF16)  # [384, 3072]
    w_o_r = np.ascontiguousarray(w_o_t.reshape(3, 128, HIDDEN).transpose(1, 0, 2))
    return {
        "x_t": x_t,
        "wqk_t": wqk_r,
        "w_o_t": w_o_r,
        "cos_t": C,
        "sin_t": Sg,
        "masks": np.ascontiguousarray(masks[0, :, 0:128]),
    }


def retile_x(x_t):
    """[3072, T] -> [T//512, 128, 24, 512] so each 512-token projection tile
    loads with one contiguous run per partition."""
    H, T = x_t.shape
    nt = T // 512
    # x_t[hh*128+p, t*512+c] -> out[t, p, hh, c]
    v = x_t.reshape(24, 128, nt, 512)
    return np.ascontiguousarray(v.transpose(2, 1, 0, 3))


# ---------------------------------------------------------------- device graph

def build_nc(S=2048, B=2, out_f32=False):
    import concourse.bass as bass
    import concourse.mybir as mybir
    import concourse.tile as tile
    from concourse import bacc
    from concourse.bass import ts, ds
    from concourse.masks import make_identity
    from contextlib import ExitStack

    dt = mybir.dt
    T = B * S                 # tokens total
    NT = T // 512             # 512-token tiles for proj
    HCH = HIDDEN // 128       # 24 hidden chunks
    NQT = S // 512            # q-tiles per batch
    NKC = S // 128            # k-chunks per batch
    SCALE = 1.0 / math.sqrt(HD)
    out_dt = dt.float32 if out_f32 else dt.bfloat16

    nc = bacc.Bacc("TRN2", target_bir_lowering=False, debug=False)

    x_t = nc.dram_tensor("x_t", [T // 512, 128, HCH, 512], dt.bfloat16,
                     kind="ExternalInput").ap()
    wqk = nc.dram_tensor("wqk_t", [128, 2 * DPC // 128, HCH, 128], dt.bfloat16,
                     kind="ExternalInput").ap()
    wot = nc.dram_tensor("w_o_t", [128, DPC // 128, HIDDEN], dt.bfloat16,
                     kind="ExternalInput").ap()
    cosd = nc.dram_tensor("cos_t", [HD, T], dt.bfloat16, kind="ExternalInput").ap()
    sind = nc.dram_tensor("sin_t", [HD, T], dt.bfloat16, kind="ExternalInput").ap()
    maskd = nc.dram_tensor("masks", [128, 128], dt.bfloat16, kind="ExternalInput").ap()
    outd = nc.dram_tensor("out", [HIDDEN, T], out_dt, kind="ExternalOutput").ap()

    with tile.TileContext(nc) as tc, ExitStack() as stk:
        # ------- persistent tiles (live across phases)
        pers = stk.enter_context(tc.tile_pool(name="pers", bufs=1))
        q_h = [pers.tile([128, T], dt.bfloat16, tag=f"q{h}", name=f"q{h}") for h in range(HPC)]
        k_h = [pers.tile([128, T], dt.bfloat16, tag=f"k{h}", name=f"k{h}") for h in range(HPC)]
        v1_h = [pers.tile([128, NKC * B, HD + 1], dt.bfloat16, tag=f"v{h}", name=f"v{h}")
                for h in range(HPC)]

        for h in range(HPC):
            nc.gpsimd.memset(q_h[h][HD:128, :], 0.0)
            nc.gpsimd.memset(k_h[h][HD:128, :], 0.0)
            nc.gpsimd.memset(v1_h[h][:, :, HD:HD + 1], 1.0)

        # ------- phase 1+2 fused: QK projection, v-transposes, RoPE —
        # interleaved per 512-token tile so PE stays dense and warm
        with tc.tile_pool(name="projw", bufs=1) as projw, \
             tc.tile_pool(name="xload", bufs=2) as xload, \
             tc.tile_pool(name="projsc", bufs=3) as projsc, \
             tc.tile_pool(name="swp", bufs=4) as swp, \
             tc.tile_pool(name="projps", bufs=2, space="PSUM") as projps, \
             tc.tile_pool(name="tps", bufs=2, space="PSUM") as tps:
            qkw_c = []
            for dcol in range(2 * DPC // 128):
                qt_ = projw.tile([128, HCH, 128], dt.bfloat16, tag=f"qkw{dcol}",
                                 name=f"qkw{dcol}")
                nc.scalar.dma_start(qt_[:], wqk[:, dcol])
                qkw_c.append(qt_)
            C_sb = projw.tile([HD, T], dt.bfloat16, tag="cos", name="C_sb")
            S_sb = projw.tile([HD, T], dt.bfloat16, tag="sin", name="S_sb")

            ident = projw.tile([128, 128], dt.bfloat16, name="ident")
            make_identity(nc, ident)

            # dcol block -> list of (head-tensor, dst_row0, src_row0, nrows)
            def segs(block):
                out = []
                r0, r1 = 128 * block, 128 * (block + 1)
                for side, dest in ((0, q_h), (1, k_h)):
                    for h in range(HPC):
                        h0 = side * DPC + h * HD
                        lo, hi = max(r0, h0), min(r1, h0 + HD)
                        if lo < hi:
                            out.append((dest[h], lo - h0, lo - r0, hi - lo))
                return out

            def emit_vt_rope(t):
                tsl = ds(t * 512, 512)
                if t == 0:
                    # C/S tables aren't needed until the first RoPE — keep them
                    # out of the startup DMA flood
                    nc.scalar.dma_start(C_sb[:], cosd)
                    nc.scalar.dma_start(S_sb[:], sind)
                # v1[token, kc, d] = k_h[d, kc*128+token] (pre-RoPE), this t-slice
                for h in range(HPC):
                    for kc in range(4 * t, 4 * t + 4):
                        tp = tps.tile([128, 128], dt.bfloat16, tag="tp", name="tp")
                        nc.tensor.transpose(tp, k_h[h][:, ts(kc, 128)], ident)
                        nc.vector.tensor_copy(v1_h[h][:, kc, 0:HD], tp[:, 0:HD])
                # RoPE this t-slice in place: v = v*C + swap48(v)*S
                for h in range(HPC):
                    for tile_ in (k_h[h], q_h[h]):
                        sw = swp.tile([HD, 512], dt.bfloat16, tag="sw", name="sw")
                        nc.sync.dma_start(sw[0:48, :], tile_[48:HD, tsl])
                        nc.sync.dma_start(sw[48:HD, :], tile_[0:48, tsl])
                        nc.vector.tensor_tensor(
                            tile_[0:HD, tsl], tile_[0:HD, tsl], C_sb[:, tsl],
                            mybir.AluOpType.mult)
                        nc.vector.tensor_tensor(
                            sw[:], sw[:], S_sb[:, tsl], mybir.AluOpType.mult)
                        nc.vector.tensor_tensor(
                            tile_[0:HD, tsl], tile_[0:HD, tsl], sw[:],
                            mybir.AluOpType.add)

            for t in range(NT):
                tsl = ds(t * 512, 512)
                x_g = []
                for g in range(3):
                    xg = xload.tile([128, 8, 512], dt.bfloat16, tag=f"x{g}",
                                    name=f"x{g}")
                    nc.sync.dma_start(xg[:], x_t[t, :, ts(g, 8), :])
                    x_g.append(xg)
                for dcol in range(2 * DPC // 128):
                    ps = projps.tile([128, 512], dt.float32, tag="pp", name="ps")
                    for hh in range(HCH):
                        nc.tensor.matmul(
                            ps, lhsT=qkw_c[dcol][:, hh, :],
                            rhs=x_g[hh // 8][:, hh % 8, :],
                            start=(hh == 0), stop=(hh == HCH - 1))
                    sc = projsc.tile([128, 512], dt.bfloat16, tag="sc", name="sc")
                    nc.scalar.copy(sc, ps)
                    for dest, d0, s0, n in segs(dcol):
                        nc.sync.dma_start(dest[d0:d0 + n, tsl], sc[s0:s0 + n, :])
                # transposes/RoPE of the PREVIOUS tile — its scatters finished a
                # full tile ago, so the PE stream never stalls on them
                if t > 0:
                    emit_vt_rope(t - 1)
            emit_vt_rope(NT - 1)

        # ------- phase 3+4: attention + o_proj per (batch, q-tile)
        with tc.tile_pool(name="attw", bufs=1) as attw, \
             tc.tile_pool(name="pbuf", bufs=6) as pbuf, \
             tc.tile_pool(name="nrm", bufs=3) as nrm, \
             tc.tile_pool(name="unp", bufs=8) as unp, \
             tc.tile_pool(name="atq", bufs=4) as atq, \
             tc.tile_pool(name="osb", bufs=3) as osb, \
             tc.tile_pool(name="nscr", bufs=3, space="DRAM") as nscr, \
             tc.tile_pool(name="sps", bufs=4, space="PSUM") as sps, \
             tc.tile_pool(name="ops", bufs=2, space="PSUM") as ops, \
             tc.tile_pool(name="ops2", bufs=2, space="PSUM") as ops2:
            w_o_sb = attw.tile([128, DPC // 128, HIDDEN], dt.bfloat16, tag="wo",
                               name="w_o_sb")
            nc.scalar.dma_start(w_o_sb[:], wot)
            mask_sb = attw.tile([128, 128], dt.bfloat16, tag="mk", name="mask_sb")
            nc.scalar.dma_start(mask_sb[:], maskd)

            def emit_oproj(g):
                gb, gqt, gat = g
                for oc in range(HIDDEN // 128):
                    ps2 = ops2.tile([128, 512], dt.float32, tag="ops2", name="ps2")
                    for ic in range(DPC // 128):
                        nc.tensor.matmul(
                            ps2, lhsT=w_o_sb[:, ic, ts(oc, 128)],
                            rhs=gat[:, ic, :],
                            start=(ic == 0), stop=(ic == DPC // 128 - 1))
                    ot = osb.tile([128, 512], out_dt, tag="ot", name="ot")
                    nc.vector.tensor_copy(ot, ps2)
                    nc.sync.dma_start(
                        outd[ts(oc, 128), ds(gb * S + gqt * 512, 512)], ot)

            pending = []
            for qt in range(NQT):
                for b in range(B):
                    at_qt = atq.tile([128, DPC // 128, 512], dt.bfloat16, tag="atq",
                                     name="at_qt")
                    sums4 = nscr.tile([HPC, 512], dt.bfloat16, tag="sums4",
                                      name="sums4")
                    un_h = []
                    for h in range(HPC):
                        q_slice = q_h[h][:, ds(b * S + qt * 512, 512)]
                        nlive = 4 * (qt + 1)
                        o_ps = ops.tile([128, 512], dt.float32, tag="ops", name="o_ps")
                        # off-diagonal chunks (full 512-wide)
                        for kc in range(4 * qt):
                            s_ps = sps.tile([128, 512], dt.float32, tag="sps",
                                            name="s_ps")
                            nc.tensor.matmul(
                                s_ps,
                                lhsT=k_h[h][:, ds(b * S + kc * 128, 128)],
                                rhs=q_slice, start=True, stop=True)
                            p_sb = pbuf.tile([128, 512], dt.bfloat16, tag="p",
                                             name="p_sb")
                            nc.scalar.activation(
                                p_sb[:], s_ps[:], mybir.ActivationFunctionType.Exp,
                                scale=SCALE)
                            nc.tensor.matmul(
                                o_ps[0:HD + 1, :],
                                lhsT=v1_h[h][:, b * NKC + kc, :],
                                rhs=p_sb,
                                start=(kc == 0), stop=False)
                        # diagonal chunks, width-restricted to valid columns
                        for j in range(4):
                            kc = 4 * qt + j
                            w = 512 - 128 * j
                            s_ps = sps.tile([128, 512], dt.float32, tag="sps",
                                            name="s_psd")
                            nc.tensor.matmul(
                                s_ps[:, 0:w],
                                lhsT=k_h[h][:, ds(b * S + kc * 128, 128)],
                                rhs=q_slice[:, 128 * j:], start=True, stop=True)
                            p_sb = pbuf.tile([128, 512], dt.bfloat16, tag="p",
                                             name="p_sbd")
                            nc.scalar.activation(
                                p_sb[:, 0:w], s_ps[:, 0:w],
                                mybir.ActivationFunctionType.Exp, scale=SCALE)
                            nc.vector.tensor_tensor(
                                p_sb[:, 0:128], p_sb[:, 0:128],
                                mask_sb[:, :], mybir.AluOpType.mult)
                            nc.tensor.matmul(
                                o_ps[0:HD + 1, 128 * j:],
                                lhsT=v1_h[h][:, b * NKC + kc, :],
                                rhs=p_sb[:, 0:w],
                                start=(kc == 0), stop=(kc == nlive - 1))
                        # copy out unnormalized attn + sums row (releases PSUM fast)
                        un = unp.tile([HD + 1, 512], dt.bfloat16, tag="un",
                                      name=f"un{h}")
                        nc.vector.tensor_copy(un[0:64, :], o_ps[0:64, :])
                        nc.scalar.copy(un[64:HD + 1, :], o_ps[64:HD + 1, :])
                        un_h.append(un)
                        nc.sync.dma_start(sums4[h:h + 1, :], un[HD:HD + 1, :])
                    # one reciprocal for all 4 heads' sums, then normalize each
                    rb4b = nrm.tile([HPC, 512], dt.bfloat16, tag="rb4b", name="rb4b")
                    nc.sync.dma_start(rb4b[:], sums4[:])
                    rb4 = nrm.tile([HPC, 512], dt.bfloat16, tag="rb4", name="rb4")
                    with nc.allow_low_precision(
                            reason="recip in bf16: scale-only error ~0.4%"):
                        nc.vector.reciprocal(rb4[:], rb4b[:])
                    rec4 = nscr.tile([HPC, 512], dt.bfloat16, tag="rec4", name="rec4")
                    nc.sync.dma_start(rec4[:], rb4[:])
                    for h in range(HPC):
                        rb = nrm.tile([HD, 512], dt.bfloat16, tag="rb", name="rb")
                        nc.sync.dma_start(
                            rb[:], rec4[h:h + 1, :].to_broadcast([HD, 512]))
                        at = nrm.tile([HD, 512], dt.bfloat16, tag="at", name="at")
                        nc.vector.tensor_tensor(
                            at, un_h[h][0:HD, :], rb[:], mybir.AluOpType.mult)
                        r0 = h * HD
                        while r0 < (h + 1) * HD:
                            blk = r0 // 128
                            n = min(128 * (blk + 1), (h + 1) * HD) - r0
                            nc.sync.dma_start(
                                at_qt[r0 - 128 * blk: r0 - 128 * blk + n, blk, :],
                                at[r0 - h * HD: r0 - h * HD + n, :])
                            r0 += n
                    # o_proj pipelined two groups behind attention so each
                    # group's norm chain has two groups of PE work as cover
                    pending.append((b, qt, at_qt))
                    if len(pending) > 3:
                        emit_oproj(pending.pop(0))
            for g in pending:
                emit_oproj(g)

    return nc


# ---------------------------------------------------------------- entry point

_NC_CACHE = {}


def _get_nc(S, B):
    key = (S, B)
    if key not in _NC_CACHE:
        nc = build_nc(S=S, B=B)
        nc.finalize()
        _NC_CACHE[key] = nc
    return _NC_CACHE[key]


def kernel(x, w_qkv, w_o, _trace=False):
    from concourse import bass_utils

    B, S, _ = x.shape
    T = B * S
    xf = np.asarray(x).reshape(T, HIDDEN)
    x_t = retile_x(np.ascontiguousarray(xf.T).astype(BF16))
    w_qkv = np.asarray(w_qkv).astype(BF16)
    w_o = np.asarray(w_o).astype(BF16)
    C, Sg = _rope_tables(S, T)
    masks = _masks()

    in_maps = [host_inputs_for_core(c, x_t, w_qkv, w_o, C, Sg, masks)
               for c in range(N_CORES)]

    nc = _get_nc(S, B)
    res = bass_utils.run_bass_kernel_spmd(
        nc, in_maps, core_ids=list(range(N_CORES)), trace=_trace)

    total = np.zeros((HIDDEN, T), dtype=np.float32)
    for c in range(N_CORES):
        total += np.asarray(res.results[c]["out"], dtype=np.float32)
    out = total.T.reshape(B, S, HIDDEN).astype(BF16)
    if _trace:
        return out, res
    return out
